# revision 15
# baseline (speedup 1.0000x reference)
"""Trainium2 Bass kernel for nn_IngredientScannerLoss.

Per row (12 coords = 6 (x,y) pairs):
    delta = output - target
    dist_j = sqrt(dx_j^2 + dy_j^2)
    n_j    = (s0_j*dx_j > 0) + (s1_j*dy_j > 0)   (sign-gated count, 0/1/2)
    f(x)   = ((x+1)^1.2 - 1)*2
    t_j    = [dist, f(dist), f(f(dist))][n_j]
    loss   = sum_j t_j

Data-parallel over 8 NeuronCores: rows split 8 x 500_000, each shard
zero-padded to 501_760 = 128*560*7 rows; tiles are [128, 560*12] fp32.

v2 design notes (measured rates on HW, cyc/elem @0.96GHz):
  - subtract: fp32 TT in-place (1.02 c/e, port-bound floor). GPSIMD/Pool
    compute is avoided entirely: concurrent Pool+DVE ops serialize
    catastrophically (measured 25x stalls).
  - squares: custom DVE op (sq+sq) reading 1D stride-2 APs = 1.05 c/e;
    2D APs cost 1.71 c/e, so s stays r-major (row-major, pair fastest).
  - values (s, dist, t, W0, W1, selects) in fp16: TS 4x (0.30),
    TT 2x (0.55); fp32 delta is kept for exact strict-sign gates
    (fp16/bf16 rounding of inputs flips gates near delta=0 and single
    corrupted rows fail rel_max).
  - gate masks n/m2 stored pair-major so gate writes are contiguous;
    copy_predicated reads masks through strided APs (stride-insensitive,
    measured).
  - ACT runs 6 full-width contiguous passes (strided ACT writes cost
    4.6 c/e -- forbidden); t2/W1 computed for all 6 pairs (pairs 4,5
    results are discarded by m2=0) because a 4-pair subset would need
    strided ACT access.
  - single act table set natural_log_exp (contains ln+exp+square) via
    the get_activation_tables patch, so no per-tile table reloads.
"""

import numpy as np

import concourse.bacc as bacc
import concourse.bass as bass
import concourse.mybir as mybir
import concourse.tile as tile
from concourse import dve_ops
from concourse.bass_utils import run_bass_kernel_spmd
from concourse.dve_ops import DveOp
from concourse.dve_spec import Spec, Src0, Src1, C0, C1, Zero, _has_src1, lower, sq
from concourse.dve_uop import DveOpSpec

P = 128
COLS = 12
NPAIR = 6
B = 4_000_000
N_CORES = 8
ROWS_VALID = B // N_CORES          # 500_000
RT = 784                           # rows per partition per tile
NT = 5                             # tiles per core
ROWS_PC = P * RT * NT              # 501_760 padded rows per core
LN2 = 0.6931471805599453

# per-coordinate condition signs (see reference _SIGNS)
SIGNS = [1.0, 1.0, 1.0, -1.0, -1.0, -1.0, -1.0, 1.0, 0.0, 1.0, 0.0, -1.0]

F32 = mybir.dt.float32
F16 = mybir.dt.float16
I16 = mybir.dt.int16
AF = mybir.ActivationFunctionType
ALU = mybir.AluOpType

# ---------------------------------------------------------------- custom ops


def _register_op(name: str, spec: Spec, subdim: bool = False) -> DveOp:
    for op in dve_ops.OPS:
        if op.name == name:
            return op
    if name not in dve_ops._SUB_OPCODE_FOR_NAME:
        row = max(dve_ops._SUB_OPCODE_FOR_NAME.values()) + 1
        assert row < 0x20, "custom DVE opcode rows exhausted"
        dve_ops._SUB_OPCODE_FOR_NAME[name] = row
    shas = {}
    for ver in ("v3", "v4"):
        try:
            shas[ver] = DveOpSpec(
                name=name,
                opcode=dve_ops.get_dve_sub_opcode(name),
                uops=lower(spec, ver=ver),
                rd1_en=_has_src1(spec),
            ).sha(ver)
        except Exception:
            pass
    op = DveOp(name, spec, subdim, shas)
    dve_ops.OPS.append(op)
    dve_ops.CUSTOM_DVE_SPECS[name] = spec
    return op


# s = in0^2 + in1^2  (in0/in1 = even/odd delta columns)
PAIRDIST = _register_op(
    "ANT_PAIRDIST",
    Spec(
        body=sq(Src0) + sq(Src1),
        reference=lambda in0, in1, s0, s1, imm2: (
            in0.astype(np.float32) ** 2 + in1.astype(np.float32) ** 2
        ),
    ),
)

# n = (in0*s0 > 0) + (in1*s1 > 0)
CGATE = _register_op(
    "ANT_CGATE",
    Spec(
        body=(Src0 * C0 > Zero) + (Src1 * C1 > Zero),
        reference=lambda in0, in1, s0, s1, imm2: (
            ((in0.astype(np.float32) * s0) > 0).astype(np.float32)
            + ((in1.astype(np.float32) * s1) > 0).astype(np.float32)
        ),
    ),
)


# ---------------------------------------------------------------- act tables
# The stock table-load pass resolves Exp -> exp_and_others and
# Ln -> natural_log, reloading ACT tables on every Ln<->Exp switch
# (~2.7us each). Restrict ln/exp membership to sets that hold BOTH so
# every activation resolves to natural_log_exp_and_others and the load
# hoists to one per kernel. Dict order (act_func_set_id) is preserved.

_GAT_REAL = None


def _gat_lnexp(arch):
    global _GAT_REAL
    from concourse.hw_specs import get_activation_tables

    if _GAT_REAL is None:
        _GAT_REAL = get_activation_tables
    tabs = _GAT_REAL(arch)
    out = {}
    for name, funcs in tabs.items():
        fs = set(funcs)
        if not (AF.Ln in fs and AF.Exp in fs):
            fs.discard(AF.Ln)
            fs.discard(AF.Exp)
        out[name] = fs
    return out


def _patch_act_tables():
    if bacc.get_activation_tables is not _gat_lnexp:
        global _GAT_REAL
        _GAT_REAL = bacc.get_activation_tables
        bacc.get_activation_tables = _gat_lnexp


# ---------------------------------------------------------------- bass build


def build_nc(rt: int = RT, nt: int = NT):
    """Build the single-core SPMD program for [P*rt*nt, 12] inputs.

    Tile schedule: the first/last tiles are half-sized to shorten the
    pipeline fill and drain (the serial dependency chain of one tile is
    ~2x its steady-state cost).
    """
    _patch_act_tables()
    rows = P * rt * nt
    rts = [rt] * nt
    w6 = rt * NPAIR          # fp16 value width (pairs)
    w12 = rt * COLS          # fp32 delta width
    nc = bacc.Bacc("TRN2", debug=False, target_bir_lowering=False,
                   num_devices=N_CORES)
    # activation biases need registered const APs (only 0.0/1.0 ship)
    for cv in (-1.0, -2.0, LN2):
        if (F32, cv) not in nc.const_aps.aps:
            ct = nc.alloc_sbuf_tensor(f"const-f32-{cv}", [P, 1], F32)
            nc.gpsimd.memset(ct.ap(), cv)
            nc.const_aps.aps[(F32, cv)] = ct.ap()
    nc.all_engine_barrier()
    a = nc.dram_tensor("output", [rows, COLS], F32, kind="ExternalInput").ap()
    b = nc.dram_tensor("target", [rows, COLS], F32, kind="ExternalInput").ap()
    o = nc.dram_tensor("loss", [rows], F32, kind="ExternalOutput").ap()

    with tile.TileContext(nc) as tc:
        with tc.tile_pool(name="sb", bufs=2) as pool:
            off = 0
            for i, rti in enumerate(rts):
                u6 = rti * NPAIR
                u12 = rti * COLS
                u4 = rti * 4
                u3 = rti * 3
                an = a[off * P:(off + rti) * P].rearrange(
                    "(p r) m -> p (r m)", p=P)
                bn = b[off * P:(off + rti) * P].rearrange(
                    "(p r) m -> p (r m)", p=P)
                on = o[off * P:(off + rti) * P].rearrange(
                    "(p r) -> p r", p=P)
                off += rti

                ta = pool.tile([P, w12], F32, tag="ta")
                nc.sync.dma_start(out=ta[:, 0:u12], in_=an)
                tb = pool.tile([P, w12], F32, tag="tb")
                nc.sync.dma_start(out=tb[:, 0:u12], in_=bn)
                # fp16 scratch carved from ta/tb: both fp32 payloads are
                # dead after the subtract (tb) / PD+gates (ta), and the
                # Tile tracker orders the overlay writes after those reads
                ta16 = ta[:].bitcast(F16)
                tb16 = tb[:].bitcast(F16)

                # ---- delta = a - b, in place on ta (fp32, exact signs)
                nc.vector.tensor_tensor(ta[:, 0:u12], ta[:, 0:u12],
                                        tb[:, 0:u12], ALU.subtract)
                delta = ta[:, 0:u12]

                # ---- s = dx^2 + dy^2, PAIR-MAJOR fp16 (j-outer 2D APs)
                d4 = delta.rearrange("p (r j two) -> p two j r",
                                     two=2, j=NPAIR)
                slt = pool.tile([P, w6], F16, tag="slt")
                nc.vector._custom_dve(PAIRDIST, out=slt[:, 0:u6],
                                      in0=d4[:, 0], in1=d4[:, 1])

                # ---- gates -> n, PAIR-MAJOR fp16 (contiguous writes)
                n16 = tb16[:, 0:w6]
                for j in range(NPAIR):
                    xs = slice(j * rti, (j + 1) * rti)
                    if SIGNS[2 * j] != 0.0:
                        nc.vector._custom_dve(
                            CGATE, out=n16[:, xs],
                            in0=d4[:, 0, j], in1=d4[:, 1, j],
                            s0=SIGNS[2 * j], s1=SIGNS[2 * j + 1],
                        )
                    else:
                        op = ALU.is_gt if SIGNS[2 * j + 1] > 0 else ALU.is_lt
                        nc.vector.tensor_scalar(n16[:, xs], d4[:, 1, j],
                                                0.0, None, op)

                # ---- ACT chain, one table set (ln+exp), all pair-major
                # contiguous; t2/W1 on the pairs-0..3 prefix. ACT also
                # takes the -2 folds (Copy w/ bias) and m2 (Relu w/ bias)
                # to offload the busier DVE:
                #   lt  = ln(s)            (in-place on slt)
                #   res = exp(0.5*lt)      = dist
                #   t   = ln(res + 1)      (in-place on slt)
                #   W0  = exp(1.2*t + ln2) = d1 + 2
                #   t2  = ln(W0 - 1)       (in-place on slt prefix)
                #   W1  = exp(1.2*t2+ln2)  = d2 + 2
                #   d1  = W0 - 2 (in-place), d2 = W1 - 2 (in-place)
                #   m2  = relu(n - 1)      (pm prefix; pairs 4,5 always 0)
                nc.scalar.activation(slt[:, 0:u6], slt[:, 0:u6], AF.Ln)
                res = pool.tile([P, w6], F16, tag="res")
                nc.scalar.activation(res[:, 0:u6], slt[:, 0:u6], AF.Exp,
                                     scale=0.5)
                nc.scalar.activation(slt[:, 0:u6], res[:, 0:u6], AF.Ln,
                                     bias=1.0)
                w0 = tb16[:, w6:2 * w6]
                nc.scalar.activation(w0[:, 0:u6], slt[:, 0:u6], AF.Exp,
                                     scale=1.2, bias=LN2)
                nc.scalar.activation(slt[:, 0:u4], w0[:, 0:u4], AF.Ln,
                                     bias=-1.0)
                w1 = ta16[:, 0:rt * 4]
                nc.scalar.activation(w1[:, 0:u4], slt[:, 0:u4], AF.Exp,
                                     scale=1.2, bias=LN2)
                nc.scalar.activation(w0[:, 0:u6], w0[:, 0:u6], AF.Copy,
                                     bias=-2.0)
                nc.scalar.activation(w1[:, 0:u4], w1[:, 0:u4], AF.Copy,
                                     bias=-2.0)
                # m2 into its own tile right after the gates (no WAR on
                # cp1's n16 read -> ACT never stalls on DVE mid-select)
                m2t = ta16[:, rt * 4:rt * 8]
                nc.scalar.activation(m2t[:, 0:u4], n16[:, 0:u4], AF.Relu,
                                     bias=-1.0)

                # ---- select: res overwritten by d1 where n>=1, d2 where
                # n=2. All APs pair-major contiguous. fp16 {0.,1.,2.}
                # bitcast int16 is nonzero exactly where the float is.
                nc.vector.copy_predicated(res[:, 0:u6],
                                          n16[:, 0:u6].bitcast(I16),
                                          w0[:, 0:u6])
                nc.vector.copy_predicated(res[:, 0:u4],
                                          m2t[:, 0:u4].bitcast(I16),
                                          w1[:, 0:u4])

                # ---- row sums: in-place contiguous fp16 tree over pairs
                nc.vector.tensor_tensor(res[:, 0:u3], res[:, 0:u3],
                                        res[:, u3:u6], ALU.add)
                nc.vector.tensor_tensor(res[:, 0:rti], res[:, 0:rti],
                                        res[:, rti:2 * rti], ALU.add)
                ot = ta[:, rt * 4:rt * 5]
                nc.vector.tensor_tensor(ot[:, 0:rti], res[:, 0:rti],
                                        res[:, 2 * rti:u3], ALU.add)
                nc.sync.dma_start(out=on, in_=ot[:, 0:rti])
    nc.compile()
    return nc


_NC_CACHE: dict = {}


def _get_nc(rt: int = RT, nt: int = NT):
    key = (rt, nt)
    if key not in _NC_CACHE:
        _NC_CACHE[key] = build_nc(rt, nt)
    return _NC_CACHE[key]


# ---------------------------------------------------------------- entrypoint


def kernel(output, target):
    a = np.asarray(output, dtype=np.float32)
    b = np.asarray(target, dtype=np.float32)
    assert a.shape == (B, COLS) and b.shape == (B, COLS)

    a_sh = np.zeros((N_CORES, ROWS_PC, COLS), dtype=np.float32)
    b_sh = np.zeros((N_CORES, ROWS_PC, COLS), dtype=np.float32)
    a_sh[:, :ROWS_VALID, :] = a.reshape(N_CORES, ROWS_VALID, COLS)
    b_sh[:, :ROWS_VALID, :] = b.reshape(N_CORES, ROWS_VALID, COLS)

    nc = _get_nc()
    in_maps = [
        {"output": a_sh[c], "target": b_sh[c]} for c in range(N_CORES)
    ]
    r = run_bass_kernel_spmd(nc, in_maps, list(range(N_CORES)))
    out = np.empty((N_CORES, ROWS_VALID), dtype=np.float32)
    for c in range(N_CORES):
        out[c] = r.results[c]["loss"][:ROWS_VALID]
    return out.reshape(B)


# revision 16
# speedup vs baseline: 1.2136x; 1.2136x over previous
"""Trainium2 Bass kernel for nn_IngredientScannerLoss.

Per row (12 coords = 6 (x,y) pairs):
    delta = output - target
    dist_j = sqrt(dx_j^2 + dy_j^2)
    n_j    = (s0_j*dx_j > 0) + (s1_j*dy_j > 0)   (sign-gated count, 0/1/2)
    f(x)   = ((x+1)^1.2 - 1)*2
    t_j    = [dist, f(dist), f(f(dist))][n_j]
    loss   = sum_j t_j

Data-parallel over 8 NeuronCores: rows split 8 x 500_000, each shard
zero-padded to 501_760 = 128*560*7 rows; tiles are [128, 560*12] fp32.

v2 design notes (measured rates on HW, cyc/elem @0.96GHz):
  - subtract: fp32 TT in-place (1.02 c/e, port-bound floor). GPSIMD/Pool
    compute is avoided entirely: concurrent Pool+DVE ops serialize
    catastrophically (measured 25x stalls).
  - squares: custom DVE op (sq+sq) reading 1D stride-2 APs = 1.05 c/e;
    2D APs cost 1.71 c/e, so s stays r-major (row-major, pair fastest).
  - values (s, dist, t, W0, W1, selects) in fp16: TS 4x (0.30),
    TT 2x (0.55); fp32 delta is kept for exact strict-sign gates
    (fp16/bf16 rounding of inputs flips gates near delta=0 and single
    corrupted rows fail rel_max).
  - gate masks n/m2 stored pair-major so gate writes are contiguous;
    copy_predicated reads masks through strided APs (stride-insensitive,
    measured).
  - ACT runs 6 full-width contiguous passes (strided ACT writes cost
    4.6 c/e -- forbidden); t2/W1 computed for all 6 pairs (pairs 4,5
    results are discarded by m2=0) because a 4-pair subset would need
    strided ACT access.
  - single act table set natural_log_exp (contains ln+exp+square) via
    the get_activation_tables patch, so no per-tile table reloads.
"""

import numpy as np

import concourse.bacc as bacc
import concourse.bass as bass
import concourse.mybir as mybir
import concourse.tile as tile
from concourse import dve_ops
from concourse.bass_utils import run_bass_kernel_spmd
from concourse.dve_ops import DveOp
from concourse.dve_spec import Spec, Src0, Src1, C0, C1, Zero, _has_src1, lower, sq
from concourse.dve_uop import DveOpSpec

P = 128
COLS = 12
NPAIR = 6
B = 4_000_000
N_CORES = 8
ROWS_VALID = B // N_CORES          # 500_000
RT = 560                           # rows per partition per tile
NT = 7                             # tiles per core
ROWS_PC = P * RT * NT              # 501_760 padded rows per core
LN2 = 0.6931471805599453

# per-coordinate condition signs (see reference _SIGNS)
SIGNS = [1.0, 1.0, 1.0, -1.0, -1.0, -1.0, -1.0, 1.0, 0.0, 1.0, 0.0, -1.0]

F32 = mybir.dt.float32
F16 = mybir.dt.float16
I16 = mybir.dt.int16
AF = mybir.ActivationFunctionType
ALU = mybir.AluOpType

# ---------------------------------------------------------------- custom ops


def _register_op(name: str, spec: Spec, subdim: bool = False) -> DveOp:
    for op in dve_ops.OPS:
        if op.name == name:
            return op
    if name not in dve_ops._SUB_OPCODE_FOR_NAME:
        row = max(dve_ops._SUB_OPCODE_FOR_NAME.values()) + 1
        assert row < 0x20, "custom DVE opcode rows exhausted"
        dve_ops._SUB_OPCODE_FOR_NAME[name] = row
    shas = {}
    for ver in ("v3", "v4"):
        try:
            shas[ver] = DveOpSpec(
                name=name,
                opcode=dve_ops.get_dve_sub_opcode(name),
                uops=lower(spec, ver=ver),
                rd1_en=_has_src1(spec),
            ).sha(ver)
        except Exception:
            pass
    op = DveOp(name, spec, subdim, shas)
    dve_ops.OPS.append(op)
    dve_ops.CUSTOM_DVE_SPECS[name] = spec
    return op


# s = in0^2 + in1^2  (in0/in1 = even/odd delta columns)
PAIRDIST = _register_op(
    "ANT_PAIRDIST",
    Spec(
        body=sq(Src0) + sq(Src1),
        reference=lambda in0, in1, s0, s1, imm2: (
            in0.astype(np.float32) ** 2 + in1.astype(np.float32) ** 2
        ),
    ),
)

# n = (in0*s0 > 0) + (in1*s1 > 0)
CGATE = _register_op(
    "ANT_CGATE",
    Spec(
        body=(Src0 * C0 > Zero) + (Src1 * C1 > Zero),
        reference=lambda in0, in1, s0, s1, imm2: (
            ((in0.astype(np.float32) * s0) > 0).astype(np.float32)
            + ((in1.astype(np.float32) * s1) > 0).astype(np.float32)
        ),
    ),
)


# ---------------------------------------------------------------- act tables
# The stock table-load pass resolves Exp -> exp_and_others and
# Ln -> natural_log, reloading ACT tables on every Ln<->Exp switch
# (~2.7us each). Restrict ln/exp membership to sets that hold BOTH so
# every activation resolves to natural_log_exp_and_others and the load
# hoists to one per kernel. Dict order (act_func_set_id) is preserved.

_GAT_REAL = None


def _gat_lnexp(arch):
    global _GAT_REAL
    from concourse.hw_specs import get_activation_tables

    if _GAT_REAL is None:
        _GAT_REAL = get_activation_tables
    tabs = _GAT_REAL(arch)
    out = {}
    for name, funcs in tabs.items():
        fs = set(funcs)
        if not (AF.Ln in fs and AF.Exp in fs):
            fs.discard(AF.Ln)
            fs.discard(AF.Exp)
        out[name] = fs
    return out


def _patch_act_tables():
    if bacc.get_activation_tables is not _gat_lnexp:
        global _GAT_REAL
        _GAT_REAL = bacc.get_activation_tables
        bacc.get_activation_tables = _gat_lnexp


# ---------------------------------------------------------------- bass build


def build_nc(rt: int = RT, nt: int = NT):
    """Build the single-core SPMD program for [P*rt*nt, 12] inputs.

    Tile schedule: the first/last tiles are half-sized to shorten the
    pipeline fill and drain (the serial dependency chain of one tile is
    ~2x its steady-state cost).
    """
    _patch_act_tables()
    rows = P * rt * nt
    half = rt // 2
    rts = [half] + [rt] * (nt - 1) + [half]
    w6 = rt * NPAIR          # fp16 value width (pairs)
    w12 = rt * COLS          # fp32 delta width
    nc = bacc.Bacc("TRN2", debug=False, target_bir_lowering=False,
                   num_devices=N_CORES)
    # activation biases need registered const APs (only 0.0/1.0 ship)
    for cv in (-1.0, -2.0, LN2):
        if (F32, cv) not in nc.const_aps.aps:
            ct = nc.alloc_sbuf_tensor(f"const-f32-{cv}", [P, 1], F32)
            nc.gpsimd.memset(ct.ap(), cv)
            nc.const_aps.aps[(F32, cv)] = ct.ap()
    nc.all_engine_barrier()
    a = nc.dram_tensor("output", [rows, COLS], F32, kind="ExternalInput").ap()
    b = nc.dram_tensor("target", [rows, COLS], F32, kind="ExternalInput").ap()
    o = nc.dram_tensor("loss", [rows], F32, kind="ExternalOutput").ap()

    with tile.TileContext(nc) as tc:
        with tc.tile_pool(name="sb", bufs=2) as pool:
            off = 0
            for i, rti in enumerate(rts):
                u6 = rti * NPAIR
                u12 = rti * COLS
                u4 = rti * 4
                u3 = rti * 3
                an = a[off * P:(off + rti) * P].rearrange(
                    "(p r) m -> p (r m)", p=P)
                bn = b[off * P:(off + rti) * P].rearrange(
                    "(p r) m -> p (r m)", p=P)
                on = o[off * P:(off + rti) * P].rearrange(
                    "(p r) -> p r", p=P)
                off += rti

                ta = pool.tile([P, w12], F32, tag="ta")
                nc.sync.dma_start(out=ta[:, 0:u12], in_=an)
                tb = pool.tile([P, w12], F32, tag="tb")
                nc.sync.dma_start(out=tb[:, 0:u12], in_=bn)

                # ---- delta = a - b, in place on ta (fp32, exact signs)
                nc.vector.tensor_tensor(ta[:, 0:u12], ta[:, 0:u12],
                                        tb[:, 0:u12], ALU.subtract)
                delta = ta[:, 0:u12]

                # ---- s = dx^2 + dy^2, PAIR-MAJOR fp16 (j-outer 2D APs)
                d4 = delta.rearrange("p (r j two) -> p two j r",
                                     two=2, j=NPAIR)
                slt = pool.tile([P, w6], F16, tag="slt")
                nc.vector._custom_dve(PAIRDIST, out=slt[:, 0:u6],
                                      in0=d4[:, 0], in1=d4[:, 1])

                # ---- gates -> n, PAIR-MAJOR fp16 (contiguous writes)
                n16 = pool.tile([P, w6], F16, tag="n16")
                for j in range(NPAIR):
                    xs = slice(j * rti, (j + 1) * rti)
                    if SIGNS[2 * j] != 0.0:
                        nc.vector._custom_dve(
                            CGATE, out=n16[:, xs],
                            in0=d4[:, 0, j], in1=d4[:, 1, j],
                            s0=SIGNS[2 * j], s1=SIGNS[2 * j + 1],
                        )
                    else:
                        op = ALU.is_gt if SIGNS[2 * j + 1] > 0 else ALU.is_lt
                        nc.vector.tensor_scalar(n16[:, xs], d4[:, 1, j],
                                                0.0, None, op)

                # ---- ACT chain, one table set (ln+exp), all pair-major
                # contiguous; t2/W1 on the pairs-0..3 prefix. ACT also
                # takes the -2 folds (Copy w/ bias) and m2 (Relu w/ bias)
                # to offload the busier DVE:
                #   lt  = ln(s)            (in-place on slt)
                #   res = exp(0.5*lt)      = dist
                #   t   = ln(res + 1)      (in-place on slt)
                #   W0  = exp(1.2*t + ln2) = d1 + 2
                #   t2  = ln(W0 - 1)       (in-place on slt prefix)
                #   W1  = exp(1.2*t2+ln2)  = d2 + 2
                #   d1  = W0 - 2 (in-place), d2 = W1 - 2 (in-place)
                #   m2  = relu(n - 1)      (pm prefix; pairs 4,5 always 0)
                nc.scalar.activation(slt[:, 0:u6], slt[:, 0:u6], AF.Ln)
                res = pool.tile([P, w6], F16, tag="res")
                nc.scalar.activation(res[:, 0:u6], slt[:, 0:u6], AF.Exp,
                                     scale=0.5)
                nc.scalar.activation(slt[:, 0:u6], res[:, 0:u6], AF.Ln,
                                     bias=1.0)
                w0 = pool.tile([P, w6], F16, tag="w0")
                nc.scalar.activation(w0[:, 0:u6], slt[:, 0:u6], AF.Exp,
                                     scale=1.2, bias=LN2)
                nc.scalar.activation(slt[:, 0:u4], w0[:, 0:u4], AF.Ln,
                                     bias=-1.0)
                w1 = pool.tile([P, rt * 4], F16, tag="w1")
                nc.scalar.activation(w1[:, 0:u4], slt[:, 0:u4], AF.Exp,
                                     scale=1.2, bias=LN2)
                nc.scalar.activation(w0[:, 0:u6], w0[:, 0:u6], AF.Copy,
                                     bias=-2.0)
                nc.scalar.activation(w1[:, 0:u4], w1[:, 0:u4], AF.Copy,
                                     bias=-2.0)
                # m2 into its own tile right after the gates (no WAR on
                # cp1's n16 read -> ACT never stalls on DVE mid-select)
                m2t = pool.tile([P, rt * 4], F16, tag="m2t")
                nc.scalar.activation(m2t[:, 0:u4], n16[:, 0:u4], AF.Relu,
                                     bias=-1.0)

                # ---- select: res overwritten by d1 where n>=1, d2 where
                # n=2. All APs pair-major contiguous. fp16 {0.,1.,2.}
                # bitcast int16 is nonzero exactly where the float is.
                nc.vector.copy_predicated(res[:, 0:u6],
                                          n16[:, 0:u6].bitcast(I16),
                                          w0[:, 0:u6])
                nc.vector.copy_predicated(res[:, 0:u4],
                                          m2t[:, 0:u4].bitcast(I16),
                                          w1[:, 0:u4])

                # ---- row sums: in-place contiguous fp16 tree over pairs
                nc.vector.tensor_tensor(res[:, 0:u3], res[:, 0:u3],
                                        res[:, u3:u6], ALU.add)
                nc.vector.tensor_tensor(res[:, 0:rti], res[:, 0:rti],
                                        res[:, rti:2 * rti], ALU.add)
                ot = pool.tile([P, rt], F32, tag="ot")
                nc.vector.tensor_tensor(ot[:, 0:rti], res[:, 0:rti],
                                        res[:, 2 * rti:u3], ALU.add)
                nc.sync.dma_start(out=on, in_=ot[:, 0:rti])
    nc.compile()
    return nc


_NC_CACHE: dict = {}


def _get_nc(rt: int = RT, nt: int = NT):
    key = (rt, nt)
    if key not in _NC_CACHE:
        _NC_CACHE[key] = build_nc(rt, nt)
    return _NC_CACHE[key]


# ---------------------------------------------------------------- entrypoint


def kernel(output, target):
    a = np.asarray(output, dtype=np.float32)
    b = np.asarray(target, dtype=np.float32)
    assert a.shape == (B, COLS) and b.shape == (B, COLS)

    a_sh = np.zeros((N_CORES, ROWS_PC, COLS), dtype=np.float32)
    b_sh = np.zeros((N_CORES, ROWS_PC, COLS), dtype=np.float32)
    a_sh[:, :ROWS_VALID, :] = a.reshape(N_CORES, ROWS_VALID, COLS)
    b_sh[:, :ROWS_VALID, :] = b.reshape(N_CORES, ROWS_VALID, COLS)

    nc = _get_nc()
    in_maps = [
        {"output": a_sh[c], "target": b_sh[c]} for c in range(N_CORES)
    ]
    r = run_bass_kernel_spmd(nc, in_maps, list(range(N_CORES)))
    out = np.empty((N_CORES, ROWS_VALID), dtype=np.float32)
    for c in range(N_CORES):
        out[c] = r.results[c]["loss"][:ROWS_VALID]
    return out.reshape(B)


# revision 17
# speedup vs baseline: 1.2144x; 1.0007x over previous
"""Trainium2 Bass kernel for nn_IngredientScannerLoss.

Per row (12 coords = 6 (x,y) pairs):
    delta = output - target
    dist_j = sqrt(dx_j^2 + dy_j^2)
    n_j    = (s0_j*dx_j > 0) + (s1_j*dy_j > 0)   (sign-gated count, 0/1/2)
    f(x)   = ((x+1)^1.2 - 1)*2
    t_j    = [dist, f(dist), f(f(dist))][n_j]
    loss   = sum_j t_j

Data-parallel over 8 NeuronCores: rows split 8 x 500_000, each shard
zero-padded to 501_760 = 128*560*7 rows; tiles are [128, 560*12] fp32.

Design notes (rates measured on HW, cyc/elem @0.96GHz):
  - subtract: fp32 TT in-place (1.02 c/e, read-port-bound floor).
    GPSIMD/Pool compute is avoided entirely: concurrent Pool+DVE ops
    serialize catastrophically (measured 25x stalls). DMA-accumulate
    (CCE) rejects subtract and wedges on SBUF destinations.
  - fp32 delta is kept for the exact strict-sign gates: fp16/bf16
    rounding of the inputs flips gates near delta=0 and a single
    corrupted row fails rel_max (select jumps by f(f(d))-f(d)).
  - everything downstream of delta is PAIR-MAJOR fp16: s/n from custom
    DVE ops with contiguous writes, ACT chain contiguous (strided ACT
    access costs 4.6 c/e -- forbidden), t2/W1 on the pairs-0..3 prefix,
    all-contiguous copy_predicated selects (cp is 1x and mask-stride
    sensitive only above ~12B strides), in-place contiguous fp16 sum
    tree. TS fp16 runs 4x (0.30 c/e), TT fp16 2x (0.55).
  - ACT absorbs the -2 folds (Copy w/ bias) and m2 = relu(n-1) (to a
    separate tile so cp1's mask read never stalls ACT); single act
    table set natural_log_exp via the get_activation_tables patch, so
    no per-tile table reloads.
  - first/last tiles are half-sized to shorten pipeline fill/drain.
    Steady state is DVE-bound at ~96% utilization (~27.5us per 560-row
    tile: sub 7.2, pairdist 6.0, gates 6.0, selects 6.2, tree 2.2).
"""

import numpy as np

import concourse.bacc as bacc
import concourse.bass as bass
import concourse.mybir as mybir
import concourse.tile as tile
from concourse import dve_ops
from concourse.bass_utils import run_bass_kernel_spmd
from concourse.dve_ops import DveOp
from concourse.dve_spec import Spec, Src0, Src1, C0, C1, Zero, _has_src1, lower, sq
from concourse.dve_uop import DveOpSpec

P = 128
COLS = 12
NPAIR = 6
B = 4_000_000
N_CORES = 8
ROWS_VALID = B // N_CORES          # 500_000
RT = 560                           # rows per partition per tile
NT = 7                             # tiles per core
ROWS_PC = P * RT * NT              # 501_760 padded rows per core
LN2 = 0.6931471805599453

# per-coordinate condition signs (see reference _SIGNS)
SIGNS = [1.0, 1.0, 1.0, -1.0, -1.0, -1.0, -1.0, 1.0, 0.0, 1.0, 0.0, -1.0]

F32 = mybir.dt.float32
F16 = mybir.dt.float16
I16 = mybir.dt.int16
AF = mybir.ActivationFunctionType
ALU = mybir.AluOpType

# ---------------------------------------------------------------- custom ops


def _register_op(name: str, spec: Spec, subdim: bool = False) -> DveOp:
    for op in dve_ops.OPS:
        if op.name == name:
            return op
    if name not in dve_ops._SUB_OPCODE_FOR_NAME:
        row = max(dve_ops._SUB_OPCODE_FOR_NAME.values()) + 1
        assert row < 0x20, "custom DVE opcode rows exhausted"
        dve_ops._SUB_OPCODE_FOR_NAME[name] = row
    shas = {}
    for ver in ("v3", "v4"):
        try:
            shas[ver] = DveOpSpec(
                name=name,
                opcode=dve_ops.get_dve_sub_opcode(name),
                uops=lower(spec, ver=ver),
                rd1_en=_has_src1(spec),
            ).sha(ver)
        except Exception:
            pass
    op = DveOp(name, spec, subdim, shas)
    dve_ops.OPS.append(op)
    dve_ops.CUSTOM_DVE_SPECS[name] = spec
    return op


# s = in0^2 + in1^2  (in0/in1 = even/odd delta columns)
PAIRDIST = _register_op(
    "ANT_PAIRDIST",
    Spec(
        body=sq(Src0) + sq(Src1),
        reference=lambda in0, in1, s0, s1, imm2: (
            in0.astype(np.float32) ** 2 + in1.astype(np.float32) ** 2
        ),
    ),
)

# n = (in0*s0 > 0) + (in1*s1 > 0)
CGATE = _register_op(
    "ANT_CGATE",
    Spec(
        body=(Src0 * C0 > Zero) + (Src1 * C1 > Zero),
        reference=lambda in0, in1, s0, s1, imm2: (
            ((in0.astype(np.float32) * s0) > 0).astype(np.float32)
            + ((in1.astype(np.float32) * s1) > 0).astype(np.float32)
        ),
    ),
)


# ---------------------------------------------------------------- act tables
# The stock table-load pass resolves Exp -> exp_and_others and
# Ln -> natural_log, reloading ACT tables on every Ln<->Exp switch
# (~2.7us each). Restrict ln/exp membership to sets that hold BOTH so
# every activation resolves to natural_log_exp_and_others and the load
# hoists to one per kernel. Dict order (act_func_set_id) is preserved.

_GAT_REAL = None


def _gat_lnexp(arch):
    global _GAT_REAL
    from concourse.hw_specs import get_activation_tables

    if _GAT_REAL is None:
        _GAT_REAL = get_activation_tables
    tabs = _GAT_REAL(arch)
    out = {}
    for name, funcs in tabs.items():
        fs = set(funcs)
        if not (AF.Ln in fs and AF.Exp in fs):
            fs.discard(AF.Ln)
            fs.discard(AF.Exp)
        out[name] = fs
    return out


def _patch_act_tables():
    if bacc.get_activation_tables is not _gat_lnexp:
        global _GAT_REAL
        _GAT_REAL = bacc.get_activation_tables
        bacc.get_activation_tables = _gat_lnexp


# ---------------------------------------------------------------- bass build


def build_nc(rt: int = RT, nt: int = NT):
    """Build the single-core SPMD program for [P*rt*nt, 12] inputs.

    Tile schedule: the first/last tiles are half-sized to shorten the
    pipeline fill and drain (the serial dependency chain of one tile is
    ~2x its steady-state cost).
    """
    _patch_act_tables()
    rows = P * rt * nt
    half = rt // 2
    rts = [half] + [rt] * (nt - 1) + [half]
    w6 = rt * NPAIR          # fp16 value width (pairs)
    w12 = rt * COLS          # fp32 delta width
    nc = bacc.Bacc("TRN2", debug=False, target_bir_lowering=False,
                   num_devices=N_CORES)
    # activation biases need registered const APs (only 0.0/1.0 ship)
    for cv in (-1.0, -2.0, LN2):
        if (F32, cv) not in nc.const_aps.aps:
            ct = nc.alloc_sbuf_tensor(f"const-f32-{cv}", [P, 1], F32)
            nc.gpsimd.memset(ct.ap(), cv)
            nc.const_aps.aps[(F32, cv)] = ct.ap()
    nc.all_engine_barrier()
    a = nc.dram_tensor("output", [rows, COLS], F32, kind="ExternalInput").ap()
    b = nc.dram_tensor("target", [rows, COLS], F32, kind="ExternalInput").ap()
    o = nc.dram_tensor("loss", [rows], F32, kind="ExternalOutput").ap()

    with tile.TileContext(nc) as tc:
        with tc.tile_pool(name="sb", bufs=2) as pool:
            off = 0
            for i, rti in enumerate(rts):
                u6 = rti * NPAIR
                u12 = rti * COLS
                u4 = rti * 4
                u3 = rti * 3
                an = a[off * P:(off + rti) * P].rearrange(
                    "(p r) m -> p (r m)", p=P)
                bn = b[off * P:(off + rti) * P].rearrange(
                    "(p r) m -> p (r m)", p=P)
                on = o[off * P:(off + rti) * P].rearrange(
                    "(p r) -> p r", p=P)
                off += rti

                ta = pool.tile([P, w12], F32, tag="ta")
                nc.sync.dma_start(out=ta[:, 0:u12], in_=an)
                tb = pool.tile([P, w12], F32, tag="tb")
                nc.sync.dma_start(out=tb[:, 0:u12], in_=bn)

                # ---- delta = a - b, in place on ta (fp32, exact signs)
                nc.vector.tensor_tensor(ta[:, 0:u12], ta[:, 0:u12],
                                        tb[:, 0:u12], ALU.subtract)
                delta = ta[:, 0:u12]

                # ---- s = dx^2 + dy^2, PAIR-MAJOR fp16 (j-outer 2D APs)
                d4 = delta.rearrange("p (r j two) -> p two j r",
                                     two=2, j=NPAIR)
                slt = pool.tile([P, w6], F16, tag="slt")
                nc.vector._custom_dve(PAIRDIST, out=slt[:, 0:u6],
                                      in0=d4[:, 0], in1=d4[:, 1])

                # ---- gates -> n, PAIR-MAJOR fp16 (contiguous writes)
                n16 = pool.tile([P, w6], F16, tag="n16")
                for j in range(NPAIR):
                    xs = slice(j * rti, (j + 1) * rti)
                    if SIGNS[2 * j] != 0.0:
                        nc.vector._custom_dve(
                            CGATE, out=n16[:, xs],
                            in0=d4[:, 0, j], in1=d4[:, 1, j],
                            s0=SIGNS[2 * j], s1=SIGNS[2 * j + 1],
                        )
                    else:
                        op = ALU.is_gt if SIGNS[2 * j + 1] > 0 else ALU.is_lt
                        nc.vector.tensor_scalar(n16[:, xs], d4[:, 1, j],
                                                0.0, None, op)

                # ---- ACT chain, one table set (ln+exp), all pair-major
                # contiguous; t2/W1 on the pairs-0..3 prefix. ACT also
                # takes the -2 folds (Copy w/ bias) and m2 (Relu w/ bias)
                # to offload the busier DVE:
                #   lt  = ln(s)            (in-place on slt)
                #   res = exp(0.5*lt)      = dist
                #   t   = ln(res + 1)      (in-place on slt)
                #   W0  = exp(1.2*t + ln2) = d1 + 2
                #   t2  = ln(W0 - 1)       (in-place on slt prefix)
                #   W1  = exp(1.2*t2+ln2)  = d2 + 2
                #   d1  = W0 - 2 (in-place), d2 = W1 - 2 (in-place)
                #   m2  = relu(n - 1)      (pm prefix; pairs 4,5 always 0)
                nc.scalar.activation(slt[:, 0:u6], slt[:, 0:u6], AF.Ln)
                res = pool.tile([P, w6], F16, tag="res")
                nc.scalar.activation(res[:, 0:u6], slt[:, 0:u6], AF.Exp,
                                     scale=0.5)
                nc.scalar.activation(slt[:, 0:u6], res[:, 0:u6], AF.Ln,
                                     bias=1.0)
                w0 = pool.tile([P, w6], F16, tag="w0")
                nc.scalar.activation(w0[:, 0:u6], slt[:, 0:u6], AF.Exp,
                                     scale=1.2, bias=LN2)
                nc.scalar.activation(slt[:, 0:u4], w0[:, 0:u4], AF.Ln,
                                     bias=-1.0)
                w1 = pool.tile([P, rt * 4], F16, tag="w1")
                nc.scalar.activation(w1[:, 0:u4], slt[:, 0:u4], AF.Exp,
                                     scale=1.2, bias=LN2)
                nc.scalar.activation(w0[:, 0:u6], w0[:, 0:u6], AF.Copy,
                                     bias=-2.0)
                nc.scalar.activation(w1[:, 0:u4], w1[:, 0:u4], AF.Copy,
                                     bias=-2.0)
                # m2 into its own tile right after the gates (no WAR on
                # cp1's n16 read -> ACT never stalls on DVE mid-select)
                m2t = pool.tile([P, rt * 4], F16, tag="m2t")
                nc.scalar.activation(m2t[:, 0:u4], n16[:, 0:u4], AF.Relu,
                                     bias=-1.0)

                # ---- select: res overwritten by d1 where n>=1, d2 where
                # n=2. All APs pair-major contiguous. fp16 {0.,1.,2.}
                # bitcast int16 is nonzero exactly where the float is.
                nc.vector.copy_predicated(res[:, 0:u6],
                                          n16[:, 0:u6].bitcast(I16),
                                          w0[:, 0:u6])
                nc.vector.copy_predicated(res[:, 0:u4],
                                          m2t[:, 0:u4].bitcast(I16),
                                          w1[:, 0:u4])

                # ---- row sums: in-place contiguous fp16 tree over pairs
                nc.vector.tensor_tensor(res[:, 0:u3], res[:, 0:u3],
                                        res[:, u3:u6], ALU.add)
                nc.vector.tensor_tensor(res[:, 0:rti], res[:, 0:rti],
                                        res[:, rti:2 * rti], ALU.add)
                ot = pool.tile([P, rt], F32, tag="ot")
                nc.vector.tensor_tensor(ot[:, 0:rti], res[:, 0:rti],
                                        res[:, 2 * rti:u3], ALU.add)
                nc.sync.dma_start(out=on, in_=ot[:, 0:rti])
    nc.compile()
    return nc


_NC_CACHE: dict = {}


def _get_nc(rt: int = RT, nt: int = NT):
    key = (rt, nt)
    if key not in _NC_CACHE:
        _NC_CACHE[key] = build_nc(rt, nt)
    return _NC_CACHE[key]


# ---------------------------------------------------------------- entrypoint


def kernel(output, target):
    a = np.asarray(output, dtype=np.float32)
    b = np.asarray(target, dtype=np.float32)
    assert a.shape == (B, COLS) and b.shape == (B, COLS)

    a_sh = np.zeros((N_CORES, ROWS_PC, COLS), dtype=np.float32)
    b_sh = np.zeros((N_CORES, ROWS_PC, COLS), dtype=np.float32)
    a_sh[:, :ROWS_VALID, :] = a.reshape(N_CORES, ROWS_VALID, COLS)
    b_sh[:, :ROWS_VALID, :] = b.reshape(N_CORES, ROWS_VALID, COLS)

    nc = _get_nc()
    in_maps = [
        {"output": a_sh[c], "target": b_sh[c]} for c in range(N_CORES)
    ]
    r = run_bass_kernel_spmd(nc, in_maps, list(range(N_CORES)))
    out = np.empty((N_CORES, ROWS_VALID), dtype=np.float32)
    for c in range(N_CORES):
        out[c] = r.results[c]["loss"][:ROWS_VALID]
    return out.reshape(B)
